# revision 19
# baseline (speedup 1.0000x reference)
"""DCT-attention kernel for Trainium2 (8 NeuronCores, batch data-parallel).

The reference applies an orthonormal DCT-II followed immediately by its
inverse over the T axis — mathematically the identity — then dense
self-attention over the C axis with 1/sqrt(32) scaling.  So the kernel
computes, for each of the B*T = 2048 independent [C=128, W=128] tiles A:

    O = softmax(A @ A.T / sqrt(32)) @ A

Key structure:
  * S = A@A.T is symmetric, so E = exp(S/sqrt(32)) is symmetric: softmax
    needs no row-max subtraction (exponents bounded by ~max||A_c||^2 /
    sqrt(32) ~ 40, safe in fp32/bf16 range) and E can be fed back to the
    PE as the stationary operand with no transpose (E.T @ A == E @ A),
    and its row sums equal its column sums.
  * MM1 runs in fp16 (error on S ~ 8e-3 abs -> ~1e-3 on exp), avoiding
    the 2-pass fp32 LOW_HIGH matmul.  E is bf16 (needs fp32 exponent
    range), MM2 is mixed bf16 x fp16.
  * A.T comes from one batched 8-tile xbar DMA transpose per group
    (3D out AP => blockwise transpose), not the PE.
  * fp32->fp16 conversion is free via a casting GPSIMD DMA load.
  * exp is batched over 4 tiles (PSUM-bank-packed MM1 outputs) to
    amortize ACT's ~300ns fixed overhead; row sums are tiny N=1
    matmuls on the PE; reciprocals batched per 8-tile group on DVE.

Sharding: batch axis B=8 across the 8 cores, 256 tiles per core.
"""

from contextlib import ExitStack

import numpy as np

import concourse.bass as bass
import concourse.mybir as mybir
import concourse.tile as tile
from concourse import bacc
from concourse.bass_utils import run_bass_kernel_spmd

B, T, C, W = 8, 256, 128, 128
N_CORES = 8
SCALE = float(1.0 / np.sqrt(32.0))
F32 = mybir.dt.float32
F16 = mybir.dt.float16
BF16 = mybir.dt.bfloat16

GROUP = 16           # tiles per DMA group
PACK = 4             # MM1 outputs packed per PSUM bank / per exp call
A_SLOTS = 4          # fp16 input groups resident
O_SLOTS = 4          # output groups resident
AT_SLOTS = 4         # transposed groups resident
E_SLOTS = 10         # exp 4-packs resident
ACT_SCALE_EVERY = 4  # every 4th output scale runs on ScalarE, rest on DVE


def build_nc() -> bass.Bass:
    n_groups = T // GROUP
    nc = bacc.Bacc("TRN2", debug=False, num_swdge_queues=4)
    x = nc.dram_tensor("X", [T, C, W], F32, kind="ExternalInput").ap()
    y = nc.dram_tensor("out", [T, C, W], F32, kind="ExternalOutput").ap()
    xg = x.rearrange("(n g) c w -> n (g c) w", g=GROUP)   # [n_groups, G*C, W]
    yg = y.rearrange("(n g) c w -> n (g c) w", g=GROUP)

    with tile.TileContext(nc) as tc, ExitStack() as ctx:
        const_pool = ctx.enter_context(tc.tile_pool(name="const", bufs=1))
        ring_pool = ctx.enter_context(tc.tile_pool(name="ring", bufs=1))
        ps = ctx.enter_context(tc.tile_pool(name="ps", bufs=2, space="PSUM"))

        bias0 = const_pool.tile([128, 1], F32)
        nc.gpsimd.memset(bias0, 0.0)
        ones16 = const_pool.tile([128, 1], F16)
        nc.gpsimd.memset(ones16, 1.0)

        a_ring = ring_pool.tile([128, A_SLOTS * GROUP * W], F16)
        at_ring = ring_pool.tile([128, AT_SLOTS * GROUP * C], F16)
        e_ring = ring_pool.tile([128, E_SLOTS * PACK * C], BF16)
        o_ring = ring_pool.tile([128, O_SLOTS * GROUP * W], F32)
        rinv_all = const_pool.tile([128, T], F32)

        for g in range(n_groups):
            ga = (g % A_SLOTS) * GROUP * W
            gt = (g % AT_SLOTS) * GROUP * C
            go = (g % O_SLOTS) * GROUP * W

            # Casting group load (fp32 DRAM -> fp16 SBUF) on the GPSIMD
            # SWDGE path, which runs in parallel with the HWDGE ring.
            a_grp = a_ring[:, ga : ga + GROUP * W]
            nc.gpsimd.dma_start(
                a_grp.rearrange("c (t w) -> c t w", t=GROUP),
                xg[g].rearrange("(t c) w -> c t w", t=GROUP),
            )

            # Batched blockwise transpose on the SP HWDGE ring:
            # out[w, t, c] = in[c, t*W + w].
            at_grp = at_ring[:, gt : gt + GROUP * C]
            nc.sync.dma_start_transpose(
                at_grp.rearrange("w (t c) -> w t c", t=GROUP), a_grp
            )

            r_ps = ps.tile([128, GROUP], F32, tag="r_ps")
            for p in range(GROUP // PACK):
                s_ps = ps.tile([128, PACK * C], F32, tag="s_ps", bufs=3)
                for j in range(PACK):
                    t = p * PACK + j
                    at = at_ring[:, gt + t * C : gt + (t + 1) * C]
                    nc.tensor.matmul(
                        s_ps[:, j * C : (j + 1) * C],
                        lhsT=at,
                        rhs=at,
                        start=True,
                        stop=True,
                    )
                # E = exp(S/sqrt(32)) for 4 tiles in one ACT op.
                i4 = g * (GROUP // PACK) + p
                ep = (i4 % E_SLOTS) * PACK * C
                e4 = e_ring[:, ep : ep + PACK * C]
                nc.scalar.activation(
                    e4,
                    s_ps,
                    mybir.ActivationFunctionType.Exp,
                    bias=bias0,
                    scale=SCALE,
                )
                # Row sums of E (= column sums, E symmetric): N=1 matmuls.
                for j in range(PACK):
                    t = p * PACK + j
                    e = e_ring[:, ep + j * C : ep + (j + 1) * C]
                    nc.tensor.matmul(
                        r_ps[:, t : t + 1],
                        lhsT=e,
                        rhs=ones16,
                        start=True,
                        stop=True,
                    )
                # Per-pack reciprocal: avoids a group-wide barrier.
                nc.vector.reciprocal(
                    rinv_all[:, g * GROUP + p * PACK : g * GROUP + (p + 1) * PACK],
                    r_ps[:, p * PACK : (p + 1) * PACK],
                )

            for p in range(GROUP // PACK):
                i4 = g * (GROUP // PACK) + p
                ep = (i4 % E_SLOTS) * PACK * C
                o_ps = ps.tile([128, PACK * W], F32, tag="o_ps", bufs=3)
                for j in range(PACK):
                    t = p * PACK + j
                    e = e_ring[:, ep + j * C : ep + (j + 1) * C]
                    a = a_ring[:, ga + t * W : ga + (t + 1) * W]
                    # O_unnorm = E.T @ A = E @ A  (mixed bf16 x fp16)
                    nc.tensor.matmul(
                        o_ps[:, j * W : (j + 1) * W],
                        lhsT=e,
                        rhs=a,
                        start=True,
                        stop=True,
                    )
                for j in range(PACK):
                    t = p * PACK + j
                    o = o_ring[:, go + t * W : go + (t + 1) * W]
                    rinv_t = rinv_all[:, g * GROUP + t : g * GROUP + t + 1]
                    o_src = o_ps[:, j * W : (j + 1) * W]
                    if t % ACT_SCALE_EVERY == ACT_SCALE_EVERY - 1:
                        nc.scalar.mul(o, o_src, rinv_t)
                    else:
                        nc.vector.tensor_scalar_mul(o, o_src, rinv_t)

            # Stores on the SP HWDGE ring (keeps late-stage waits off ACT).
            nc.sync.dma_start(
                yg[g].rearrange("(t c) w -> c t w", t=GROUP),
                o_ring[:, go : go + GROUP * W].rearrange("c (t w) -> c t w", t=GROUP),
            )

    nc.compile()
    return nc


_NC_CACHE: dict[str, bass.Bass] = {}


def _get_nc() -> bass.Bass:
    if "nc" not in _NC_CACHE:
        _NC_CACHE["nc"] = build_nc()
    return _NC_CACHE["nc"]


def run(X: np.ndarray, **spmd_kwargs):
    """Shard over batch, run on 8 cores, gather.  Returns (output, results)."""
    assert X.shape == (B, T, C, W), X.shape
    nc = _get_nc()
    in_maps = [{"X": np.ascontiguousarray(X[i])} for i in range(N_CORES)]
    res = run_bass_kernel_spmd(nc, in_maps, list(range(N_CORES)), **spmd_kwargs)
    out = np.stack([res.results[i]["out"] for i in range(N_CORES)], axis=0)
    return out.astype(np.float32), res


def kernel(X: np.ndarray) -> np.ndarray:
    out, _ = run(np.asarray(X, dtype=np.float32))
    return out
